# revision 2
# baseline (speedup 1.0000x reference)
"""v4: time-segmented parallel recurrence (contraction-washout restart).

The per-step serial chain (~1.7us: DVE handoff -> PE mm1 -> Act exp -> PE
mm2) bounds wall time at T*L when the whole sequence is one chain. But the
recurrence is strongly contractive: a restart from zero state converges to
the true trajectory EXACTLY (fp32 bitwise) within ~32 steps. So each core
splits its T=2048 into P=4 overlapping segments:

  seg0 [0,576)  seg1 [512,1088)  seg2 [1024,1600)  seg3 [1536,2048)

segs 1-3 warm up for 64 steps from zero state, then their outputs are
exact. The 4 segments are independent chains, so the Act engine (the
throughput ceiling: one [128,448] exp per segment-step, ~558 ns busy)
stays saturated instead of idling on chain latency:
  wall ~= 576 * 4 * 558ns ~= 1.3 ms  (vs 2048 * 1700ns = 3.5 ms single-chain)

Per-segment machinery is the v3 design with H=1 (full 64-batch per
segment): one fused bf16 mm1 (K=83: s, s^2, split-bf16 x-rows), single
exp, bf16 mm2, DVE copy+square handoff, mega-tile staging, row-0 output
DMA harvesting.
"""

import numpy as np
import ml_dtypes

N_CORES = 8
CM = 800
CM_PAD = 896
NCHUNK = 7
KROWS = 83
NSTATE = 16
STAGE = 64
WARM = 64

_PROGRAM_CACHE = {}


def _seg_spans(T):
    # (gstart, length, warm) per segment; real outputs [gstart+warm, gstart+len)
    if T == 2048:
        return [(0, 576, 0), (512, 576, 64), (1024, 576, 64), (1536, 512, 64)]
    # generic fallback: quarter segments + warmup (dev/small-T use)
    q = T // 4
    spans = [(0, q, 0)]
    for p in range(1, 4):
        w = min(WARM, q)
        spans.append((p * q - w, q + w, w))
    return spans


def _build_program(B_local, T):
    import concourse.bass as bass
    import concourse.bacc as bacc
    import concourse.tile as tile
    from concourse import mybir
    from contextlib import ExitStack

    f32 = mybir.dt.float32
    bf16 = mybir.dt.bfloat16
    Act = mybir.ActivationFunctionType

    BL = B_local
    spans = _seg_spans(T)
    P = len(spans)

    nc = bacc.Bacc("TRN2", target_bir_lowering=False, debug=False)
    W_d = nc.dram_tensor("Wk", [KROWS, CM_PAD], bf16, kind="ExternalInput")
    A2_d = nc.dram_tensor("A2e", [CM_PAD, NSTATE], bf16, kind="ExternalInput")
    R_d = nc.dram_tensor("Rt", [19, T * BL], bf16, kind="ExternalInput")
    O_d = nc.dram_tensor("O1", [1, T * BL], bf16, kind="ExternalOutput")

    with tile.TileContext(nc) as tc, ExitStack() as ctx:
        singles = ctx.enter_context(tc.tile_pool(name="singles", bufs=1))
        epools = [
            ctx.enter_context(tc.tile_pool(name=f"e{g}", bufs=1, space="PSUM"))
            for g in range(P)
        ]
        spools = [
            ctx.enter_context(tc.tile_pool(name=f"s{g}", bufs=1, space="PSUM"))
            for g in range(P)
        ]

        Wsb = singles.tile([KROWS, CM_PAD], bf16)
        nc.sync.dma_start(out=Wsb, in_=W_d[:, :])
        A2sb = singles.tile([128, NCHUNK, NSTATE], bf16)
        nc.sync.dma_start(out=A2sb, in_=A2_d.rearrange("(c p) n -> p c n", p=128))

        mega = [
            [singles.tile([KROWS, STAGE * BL], bf16, name=f"mg{g}_{j}") for j in range(2)]
            for g in range(P)
        ]
        for g in range(P):
            gstart, tlen, warm = spans[g]
            for j in range(2):
                nc.gpsimd.memset(mega[g][j][0:64, :], 0.0)
                if j < tlen // STAGE:
                    nc.sync.dma_start(
                        out=mega[g][j][64:KROWS, :],
                        in_=R_d[:, (gstart + j * STAGE) * BL : (gstart + (j + 1) * STAGE) * BL],
                    )

        psis = [singles.tile([128, NCHUNK * BL], bf16, name=f"psi{g}") for g in range(P)]
        Et = [epools[g].tile([128, NCHUNK * BL], f32, name=f"E{g}") for g in range(P)]
        sp = [spools[g].tile([NSTATE, BL], f32, name=f"sp{g}") for g in range(P)]

        def front(g, t):
            gstart, tlen, warm = spans[g]
            si, ti = divmod(t, STAGE)
            if ti == 0 and si + 1 < tlen // STAGE:
                nc.sync.dma_start(
                    out=mega[g][(si + 1) % 2][64:KROWS, :],
                    in_=R_d[
                        :,
                        (gstart + (si + 1) * STAGE) * BL : (gstart + (si + 2) * STAGE) * BL,
                    ],
                )
            rv = mega[g][si % 2][0:KROWS, ti * BL : (ti + 1) * BL]
            for k in range(NCHUNK):
                nc.tensor.matmul(
                    Et[g][:, k * BL : (k + 1) * BL],
                    lhsT=Wsb[:, k * 128 : (k + 1) * 128],
                    rhs=rv,
                    start=True,
                    stop=True,
                )
            nc.scalar.activation(out=psis[g], in_=Et[g], func=Act.Exp)

        def back(g, t):
            gstart, tlen, warm = spans[g]
            for k in range(NCHUNK):
                nc.tensor.matmul(
                    sp[g],
                    lhsT=A2sb[:, k, :],
                    rhs=psis[g][:, k * BL : (k + 1) * BL],
                    start=(k == 0),
                    stop=(k == NCHUNK - 1),
                    skip_group_check=True,
                )
            w = t + 1
            mw = mega[g][(w // STAGE) % 2]
            cc = (w % STAGE) * BL
            cp = mw[0:NSTATE, cc : cc + BL]
            nc.vector.tensor_copy(out=cp, in_=sp[g])
            nc.vector.tensor_mul(mw[32:48, cc : cc + BL], cp, cp)
            if (w + 1) % STAGE == 0:
                # seg-local stage si_o = (w+1)//STAGE - 1 fully written;
                # its windows [si_o*64, (si_o+1)*64) hold outputs
                # t_out = gstart + window - 1; emit only real (>= gstart+warm)
                si_o = (w + 1) // STAGE - 1
                mt = mega[g][si_o % 2]
                lo = si_o * STAGE  # first window of the stage
                first_real_w = max(1, warm + 1) if si_o * STAGE < max(1, warm + 1) else lo
                # windows < first_real_w are warmup/nonexistent outputs
                skip = max(0, max(1, warm + 1) - lo)
                if skip < STAGE:
                    nc.sync.dma_start(
                        out=O_d[
                            :,
                            (gstart + lo + skip - 1) * BL : (gstart + lo + STAGE - 1) * BL,
                        ],
                        in_=mt[0:1, skip * BL :],
                    )

        # final window per segment: w = tlen holds output gstart + tlen - 1
        def seg_final(g):
            gstart, tlen, warm = spans[g]
            nc.sync.dma_start(
                out=O_d[:, (gstart + tlen - 1) * BL : (gstart + tlen) * BL],
                in_=mega[g][(tlen // STAGE) % 2][0:1, 0:BL],
            )

        maxT = max(s[1] for s in spans)
        for t in range(maxT + 1):
            for g in range(P):
                tlen = spans[g][1]
                if 1 <= t <= tlen:
                    back(g, t - 1)
                    if t == tlen:
                        seg_final(g)
                if t < tlen:
                    front(g, t)

    nc.compile()
    return nc


def _host_precompute(x, S, U, A, W1, b1, W2, b2):
    bf = ml_dtypes.bfloat16

    def q(a):
        return a.astype(bf).astype(np.float32)

    B, T = x.shape
    C, M, N = S.shape
    B_local = B // N_CORES

    perm = np.r_[N - 1, np.arange(N - 1)]
    Sf = S.reshape(C * M, N).astype(np.float32)
    Uf = U.reshape(C * M).astype(np.float32)
    C1 = (Sf * Sf).sum(1) + Uf * Uf
    A2e = np.zeros((CM_PAD, N), bf)
    A2e[:CM] = ((A.reshape(C * M, N) * np.exp(-C1)[:, None])[:, perm]).astype(bf)

    twoU = 2.0 * Uf
    Uh = q(twoU)
    Ul = twoU - Uh

    Wk = np.zeros((KROWS, CM_PAD), np.float32)
    Wk[0:N, :CM] = 2.0 * Sf.T[perm]
    Wk[32:48, :CM] = -1.0
    Wk[64, :CM] = Uh
    Wk[65, :CM] = Uh
    Wk[66, :CM] = Ul
    for c in range(C):
        Wk[67 + c, c * M : (c + 1) * M] = 1.0
        Wk[75 + c, c * M : (c + 1) * M] = 1.0
    Wk = Wk.astype(bf)

    h = np.maximum(x[..., None] * W1[0] + b1, 0.0)
    g = h @ W2 + b2
    g = g - g.max(-1, keepdims=True)
    lg = (g - np.log(np.exp(g).sum(-1, keepdims=True))).astype(np.float32)

    d = lg - (x * x)[..., None]
    xh = q(x)
    xl = x - xh
    dh = q(d)
    dl = d - dh

    R = np.empty((N_CORES, 19, T, B_local), bf)
    for i in range(N_CORES):
        bs = slice(i * B_local, (i + 1) * B_local)
        R[i, 0] = xh[bs].T.astype(bf)
        R[i, 1] = xl[bs].T.astype(bf)
        R[i, 2] = xh[bs].T.astype(bf)
        R[i, 3:11] = dh[bs].transpose(2, 1, 0).astype(bf)
        R[i, 11:19] = dl[bs].transpose(2, 1, 0).astype(bf)
    R = R.reshape(N_CORES, 19, T * B_local)
    return Wk, A2e, R


def kernel(x, S, U, A, W1, b1, W2, b2, T_override=None):
    x = np.asarray(x, np.float32)
    if T_override is not None:
        x = x[:, :T_override]
    B, T = x.shape
    assert B % N_CORES == 0 and T % (4 * STAGE) == 0
    B_local = B // N_CORES

    Wk, A2e, R = _host_precompute(
        np.asarray(x), np.asarray(S), np.asarray(U), np.asarray(A),
        np.asarray(W1), np.asarray(b1), np.asarray(W2), np.asarray(b2),
    )

    key = (B_local, T)
    if key not in _PROGRAM_CACHE:
        _PROGRAM_CACHE[key] = _build_program(B_local, T)
    nc = _PROGRAM_CACHE[key]

    from concourse.bass_utils import run_bass_kernel_spmd

    in_maps = [
        {"Wk": Wk, "A2e": A2e, "Rt": np.ascontiguousarray(R[i])}
        for i in range(N_CORES)
    ]
    res = run_bass_kernel_spmd(nc, in_maps, core_ids=list(range(N_CORES)))
    out = np.empty((B, T), np.float32)
    for i in range(N_CORES):
        O1 = np.asarray(res.results[i]["O1"]).astype(np.float32).reshape(T, B_local)
        out[i * B_local : (i + 1) * B_local] = O1.T
    return out


# revision 3
# speedup vs baseline: 1.0215x; 1.0215x over previous
"""v4: time-segmented parallel recurrence (contraction-washout restart).

The per-step serial chain (~1.7us: DVE handoff -> PE mm1 -> Act exp -> PE
mm2) bounds wall time at T*L when the whole sequence is one chain. But the
recurrence is strongly contractive: a restart from zero state converges to
the true trajectory EXACTLY (fp32 bitwise) within ~32 steps. So each core
splits its T=2048 into P=4 overlapping segments:

  seg0 [0,576)  seg1 [512,1088)  seg2 [1024,1600)  seg3 [1536,2048)

segs 1-3 warm up for 64 steps from zero state, then their outputs are
exact. The 4 segments are independent chains, so the Act engine (the
throughput ceiling: one [128,448] exp per segment-step, ~558 ns busy)
stays saturated instead of idling on chain latency:
  wall ~= 576 * 4 * 558ns ~= 1.3 ms  (vs 2048 * 1700ns = 3.5 ms single-chain)

Per-segment machinery is the v3 design with H=1 (full 64-batch per
segment): one fused bf16 mm1 (K=83: s, s^2, split-bf16 x-rows), single
exp, bf16 mm2, DVE copy+square handoff, mega-tile staging, row-0 output
DMA harvesting.
"""

import numpy as np
import ml_dtypes

N_CORES = 8
CM = 800
CM_PAD = 896
NCHUNK = 7
KROWS = 83
NSTATE = 16
STAGE = 56
WARM = 64

_PROGRAM_CACHE = {}


def _seg_spans(T):
    # (gstart, length, warm) per segment; real outputs [gstart+warm, gstart+len)
    if T == 2048:
        # balanced: every segment exactly 560 steps (10 stages of 56);
        # real outputs tile [0,560) [560,1056) [1056,1552) [1552,2048)
        return [(0, 560, 0), (496, 560, 64), (992, 560, 64), (1488, 560, 64)]
    # generic fallback: quarter segments + warmup (dev/small-T use)
    q = T // 4
    spans = [(0, q, 0)]
    for p in range(1, 4):
        w = min(WARM, q)
        spans.append((p * q - w, q + w, w))
    return spans


def _build_program(B_local, T):
    import concourse.bass as bass
    import concourse.bacc as bacc
    import concourse.tile as tile
    from concourse import mybir
    from contextlib import ExitStack

    f32 = mybir.dt.float32
    bf16 = mybir.dt.bfloat16
    Act = mybir.ActivationFunctionType

    BL = B_local
    spans = _seg_spans(T)
    P = len(spans)

    nc = bacc.Bacc("TRN2", target_bir_lowering=False, debug=False)
    W_d = nc.dram_tensor("Wk", [KROWS, CM_PAD], bf16, kind="ExternalInput")
    A2_d = nc.dram_tensor("A2e", [CM_PAD, NSTATE], bf16, kind="ExternalInput")
    R_d = nc.dram_tensor("Rt", [19, T * BL], bf16, kind="ExternalInput")
    O_d = nc.dram_tensor("O1", [1, T * BL], bf16, kind="ExternalOutput")

    with tile.TileContext(nc) as tc, ExitStack() as ctx:
        singles = ctx.enter_context(tc.tile_pool(name="singles", bufs=1))
        epools = [
            ctx.enter_context(tc.tile_pool(name=f"e{g}", bufs=1, space="PSUM"))
            for g in range(P)
        ]
        spools = [
            ctx.enter_context(tc.tile_pool(name=f"s{g}", bufs=1, space="PSUM"))
            for g in range(P)
        ]

        Wsb = singles.tile([KROWS, CM_PAD], bf16)
        nc.sync.dma_start(out=Wsb, in_=W_d[:, :])
        A2sb = singles.tile([128, NCHUNK, NSTATE], bf16)
        nc.sync.dma_start(out=A2sb, in_=A2_d.rearrange("(c p) n -> p c n", p=128))

        mega = [
            [singles.tile([KROWS, STAGE * BL], bf16, name=f"mg{g}_{j}") for j in range(2)]
            for g in range(P)
        ]
        for g in range(P):
            gstart, tlen, warm = spans[g]
            for j in range(2):
                nc.gpsimd.memset(mega[g][j][0:64, :], 0.0)
                if j < tlen // STAGE:
                    nc.sync.dma_start(
                        out=mega[g][j][64:KROWS, :],
                        in_=R_d[:, (gstart + j * STAGE) * BL : (gstart + (j + 1) * STAGE) * BL],
                    )

        psis = [singles.tile([128, NCHUNK * BL], bf16, name=f"psi{g}") for g in range(P)]
        Et = [epools[g].tile([128, NCHUNK * BL], f32, name=f"E{g}") for g in range(P)]
        sp = [spools[g].tile([NSTATE, BL], f32, name=f"sp{g}") for g in range(P)]

        def front(g, t):
            gstart, tlen, warm = spans[g]
            si, ti = divmod(t, STAGE)
            if ti == 0 and si + 1 < tlen // STAGE:
                nc.sync.dma_start(
                    out=mega[g][(si + 1) % 2][64:KROWS, :],
                    in_=R_d[
                        :,
                        (gstart + (si + 1) * STAGE) * BL : (gstart + (si + 2) * STAGE) * BL,
                    ],
                )
            rv = mega[g][si % 2][0:KROWS, ti * BL : (ti + 1) * BL]
            for k in range(NCHUNK):
                nc.tensor.matmul(
                    Et[g][:, k * BL : (k + 1) * BL],
                    lhsT=Wsb[:, k * 128 : (k + 1) * 128],
                    rhs=rv,
                    start=True,
                    stop=True,
                )
            nc.scalar.activation(out=psis[g], in_=Et[g], func=Act.Exp)

        def back(g, t):
            gstart, tlen, warm = spans[g]
            for k in range(NCHUNK):
                nc.tensor.matmul(
                    sp[g],
                    lhsT=A2sb[:, k, :],
                    rhs=psis[g][:, k * BL : (k + 1) * BL],
                    start=(k == 0),
                    stop=(k == NCHUNK - 1),
                    skip_group_check=True,
                )
            w = t + 1
            mw = mega[g][(w // STAGE) % 2]
            cc = (w % STAGE) * BL
            cp = mw[0:NSTATE, cc : cc + BL]
            nc.vector.tensor_copy(out=cp, in_=sp[g])
            nc.vector.tensor_mul(mw[32:48, cc : cc + BL], cp, cp)
            if (w + 1) % STAGE == 0:
                # seg-local stage si_o = (w+1)//STAGE - 1 fully written;
                # its windows [si_o*64, (si_o+1)*64) hold outputs
                # t_out = gstart + window - 1; emit only real (>= gstart+warm)
                si_o = (w + 1) // STAGE - 1
                mt = mega[g][si_o % 2]
                lo = si_o * STAGE  # first window of the stage
                first_real_w = max(1, warm + 1) if si_o * STAGE < max(1, warm + 1) else lo
                # windows < first_real_w are warmup/nonexistent outputs
                skip = max(0, max(1, warm + 1) - lo)
                if skip < STAGE:
                    nc.sync.dma_start(
                        out=O_d[
                            :,
                            (gstart + lo + skip - 1) * BL : (gstart + lo + STAGE - 1) * BL,
                        ],
                        in_=mt[0:1, skip * BL :],
                    )

        # final window per segment: w = tlen holds output gstart + tlen - 1
        def seg_final(g):
            gstart, tlen, warm = spans[g]
            nc.sync.dma_start(
                out=O_d[:, (gstart + tlen - 1) * BL : (gstart + tlen) * BL],
                in_=mega[g][(tlen // STAGE) % 2][0:1, 0:BL],
            )

        maxT = max(s[1] for s in spans)
        for t in range(maxT + 1):
            for g in range(P):
                tlen = spans[g][1]
                if 1 <= t <= tlen:
                    back(g, t - 1)
                    if t == tlen:
                        seg_final(g)
                if t < tlen:
                    front(g, t)

    nc.compile()
    return nc


def _host_precompute(x, S, U, A, W1, b1, W2, b2):
    bf = ml_dtypes.bfloat16

    def q(a):
        return a.astype(bf).astype(np.float32)

    B, T = x.shape
    C, M, N = S.shape
    B_local = B // N_CORES

    perm = np.r_[N - 1, np.arange(N - 1)]
    Sf = S.reshape(C * M, N).astype(np.float32)
    Uf = U.reshape(C * M).astype(np.float32)
    C1 = (Sf * Sf).sum(1) + Uf * Uf
    A2e = np.zeros((CM_PAD, N), bf)
    A2e[:CM] = ((A.reshape(C * M, N) * np.exp(-C1)[:, None])[:, perm]).astype(bf)

    twoU = 2.0 * Uf
    Uh = q(twoU)
    Ul = twoU - Uh

    Wk = np.zeros((KROWS, CM_PAD), np.float32)
    Wk[0:N, :CM] = 2.0 * Sf.T[perm]
    Wk[32:48, :CM] = -1.0
    Wk[64, :CM] = Uh
    Wk[65, :CM] = Uh
    Wk[66, :CM] = Ul
    for c in range(C):
        Wk[67 + c, c * M : (c + 1) * M] = 1.0
        Wk[75 + c, c * M : (c + 1) * M] = 1.0
    Wk = Wk.astype(bf)

    h = np.maximum(x[..., None] * W1[0] + b1, 0.0)
    g = h @ W2 + b2
    g = g - g.max(-1, keepdims=True)
    lg = (g - np.log(np.exp(g).sum(-1, keepdims=True))).astype(np.float32)

    d = lg - (x * x)[..., None]
    xh = q(x)
    xl = x - xh
    dh = q(d)
    dl = d - dh

    R = np.empty((N_CORES, 19, T, B_local), bf)
    for i in range(N_CORES):
        bs = slice(i * B_local, (i + 1) * B_local)
        R[i, 0] = xh[bs].T.astype(bf)
        R[i, 1] = xl[bs].T.astype(bf)
        R[i, 2] = xh[bs].T.astype(bf)
        R[i, 3:11] = dh[bs].transpose(2, 1, 0).astype(bf)
        R[i, 11:19] = dl[bs].transpose(2, 1, 0).astype(bf)
    R = R.reshape(N_CORES, 19, T * B_local)
    return Wk, A2e, R


def kernel(x, S, U, A, W1, b1, W2, b2, T_override=None):
    x = np.asarray(x, np.float32)
    if T_override is not None:
        x = x[:, :T_override]
    B, T = x.shape
    assert B % N_CORES == 0 and (T == 2048 or T % (4 * STAGE) == 0)
    B_local = B // N_CORES

    Wk, A2e, R = _host_precompute(
        np.asarray(x), np.asarray(S), np.asarray(U), np.asarray(A),
        np.asarray(W1), np.asarray(b1), np.asarray(W2), np.asarray(b2),
    )

    key = (B_local, T)
    if key not in _PROGRAM_CACHE:
        _PROGRAM_CACHE[key] = _build_program(B_local, T)
    nc = _PROGRAM_CACHE[key]

    from concourse.bass_utils import run_bass_kernel_spmd

    in_maps = [
        {"Wk": Wk, "A2e": A2e, "Rt": np.ascontiguousarray(R[i])}
        for i in range(N_CORES)
    ]
    res = run_bass_kernel_spmd(nc, in_maps, core_ids=list(range(N_CORES)))
    out = np.empty((B, T), np.float32)
    for i in range(N_CORES):
        O1 = np.asarray(res.results[i]["O1"]).astype(np.float32).reshape(T, B_local)
        out[i * B_local : (i + 1) * B_local] = O1.T
    return out
